# revision 1
# baseline (speedup 1.0000x reference)
"""Trainium2 Bass kernel for nn_DemandMap (histogram_binning).

Math: the scatter-add histogram is a dense separable 8x8 block reduction.
Site (i,j) of type t contributes ox(t, i%8)*oy(t, j%8) area terms to bins
(i//8 (+1), j//8 (+1)), so per type:  hist_t = WX_t^T @ mask_t @ WY_t
with banded weight matrices that depend only on (residue, type).

Device dataflow per core (j on partitions, i on free axis):
  smapT[jp, r] int16 (j zero-padded by 8 at top) --DMA--> [128, 512] tiles;
  35 j-tiles at stride 120 (8-row y-halo) so each tile fully owns 15 y-bins.
  masks: m_t = (st == t+1) in bf16          (DVE tensor_scalar + gpsimd)
  stage A (y-reduce): PE matmul psA[32t + (0..29), i] = WY_t^T @ m_t,
       WY_t = [128, 32] bf16 = 15 hi + 15 lo + 2 zero cols (hi/lo split of
       the f32 weights keeps fp32-level accuracy at bf16 matmul speed)
  drain psA [96, 512] f32 -> AD[g] [96, 1536] bf16 (ScalarE copy, cast)
  one xbar transpose per 3-tile group: [96, 1536] -> [128, 12, 96] into a
       per-4-group ATC tensor (so stage B sees a uniform 384-stride layout)
  stage B (x-reduce): PE matmul psB[65, 30*JL+q] += WXhi^T @ ATC
       (65th row = spill into the next core's first bin; replaces an x-halo)
       + WXlo^T @ hi-cols onto lo-cols when weights are not bf16-exact
  merge hi+lo (DVE), assemble 64-hist via ScalarE Copy(scale=-1, bias=64),
  DMA out [4, 65, 512].

Sharding: core c owns bins bx in [64c, 64c+64) and reads site rows
[512c, 512c+512) only.  The kx=1 spill that crosses the core boundary is
returned as output row 64 and added into the next core's first bin column
on the host ((64-h0) + (64-h1) - 64 = 64 - h0 - h1).
"""

import json
import os

import numpy as np
import ml_dtypes

BF16 = ml_dtypes.bfloat16

NCORES = 8
RPAD = 512  # site rows per core (i axis) — exactly 4 transpose chunks
IPAD = 512
JPAD = 4224  # 8 top zero pad + 4096 + tail pad, j axis (partition source)
NT = 35  # j-tiles, stride 120, each owns 15 y-bins
GROUPS = [list(range(6 * g, 6 * g + 6)) for g in range(4)] + [
    [24, 25, 26, 27, 28],
    [29, 30, 31],
    [32, 33, 34],
]
GCHUNKS = [[0, 1], [2, 3], [4, 5, 6]]
# JL offset of each group inside its chunk's ATC tensor
GOFF = {0: 0, 1: 6, 2: 0, 3: 6, 4: 0, 5: 5, 6: 8}
NAD = int(os.environ.get("KERNEL_NAD", "4"))  # rotating A-drain buffers
NIN_DMA = int(os.environ.get("KERNEL_NIN", "6"))  # input loaded in this many big DMAs


def _nbins(T):
    return 15 if T < 34 else 2


_PROG_CACHE = {}
_WSPLIT_DONE = [False]


def _install_wait_split():
    """This walrus build accepts only ONE sync wait per instruction; Tile
    attaches N.  Rewrite the BIR JSON: hoist all-but-one wait onto fresh
    same-engine EventSemaphore waits inserted before the offender."""
    if _WSPLIT_DONE[0]:
        return
    import concourse.bass as bass

    orig = bass.Bass.to_json_bytes

    def split(self, *a, **k):
        m = json.loads(orig(self, *a, **k))
        n = [0]
        for fn in m["functions"]:
            for blk in fn["blocks"]:
                out = []
                for ins in blk["instructions"]:
                    si = ins.get("sync_info")
                    waits = si.get("on_wait") if si else None
                    if waits and len(waits) > 1:
                        for w in waits[:-1]:
                            n[0] += 1
                            nop = {
                                "engine": ins["engine"],
                                "ins": [],
                                "outs": [],
                                "name": f"WSPLIT-{n[0]}",
                                "opcode": "EventSemaphore",
                                "sync_info": {"on_update": [], "on_wait": [w]},
                            }
                            if "debug" in ins:
                                nop["debug"] = ins["debug"]
                            out.append(nop)
                        si["on_wait"] = [waits[-1]]
                    out.append(ins)
                blk["instructions"] = out
        return json.dumps(m).encode()

    bass.Bass.to_json_bytes = split
    _WSPLIT_DONE[0] = True


def _oxy_weights(size_f32):
    """Per-residue overlap weights, matching the reference f32 formulas."""
    r = np.arange(8, dtype=np.float32)
    o0 = np.maximum(np.minimum(np.float32(8.0) - r, size_f32), np.float32(0.0))
    o1 = np.maximum(
        np.minimum(np.float32(16.0) - r, size_f32) - (np.float32(8.0) - r),
        np.float32(0.0),
    )
    return o0.astype(np.float32), o1.astype(np.float32)


def _build_wy(sy):
    """f32 [128, 3, 15]: y-stage stationary (y-halo tile form), types 1..3.
    Tile partition p holds padded j = 120T + p (jp = j + 8), so ry = p % 8;
    bin q gets ky=0 rows p//8 == q+1 and ky=1 rows p//8 == q."""
    W = np.zeros((128, 3, 15), np.float32)
    for tp in range(3):
        o0, o1 = _oxy_weights(np.float32(sy[tp + 1]))
        for p in range(128):
            if 0 <= p // 8 - 1 < 15:
                W[p, tp, p // 8 - 1] += o0[p % 8]
            if p // 8 < 15:
                W[p, tp, p // 8] += o1[p % 8]
    return W


def _build_wx(sx):
    """f32 [IPAD, 3, 65]: x-stage stationary.  Local row r = i - 512c; col 64
    is the spill bin (kx=1 of the last 8 rows -> next core's first bin)."""
    W = np.zeros((IPAD, 3, 65), np.float32)
    for tp in range(3):
        o0, o1 = _oxy_weights(np.float32(sx[tp + 1]))
        for r in range(512):
            W[r, tp, r // 8] += o0[r % 8]
            W[r, tp, r // 8 + 1] += o1[r % 8]
    return W


def _hi_lo(w):
    hi = w.astype(BF16)
    lo = (w - hi.astype(np.float32)).astype(BF16)
    return hi, lo


def _build_program(use_xlo, use_ylo):
    _install_wait_split()
    import os as _os
    import concourse.bass as bass
    import concourse.mybir as mybir
    from concourse.tile import TileContext
    from contextlib import ExitStack

    bufs_st = int(_os.environ.get("KERNEL_BUFS_ST", "6"))
    bufs_m = int(_os.environ.get("KERNEL_BUFS_M", "4"))
    bufs_pa = int(_os.environ.get("KERNEL_BUFS_PA", "3"))
    nmask_dve = int(_os.environ.get("KERNEL_MASK_DVE", "2"))
    ndrain_dve = int(_os.environ.get("KERNEL_DRAIN_DVE", "0"))

    dt = mybir.dt
    nc = bass.Bass()
    # host-packed tiles: smapT[p, 512*T + r] = site_type[120*T + p - 8, r]
    smapT = nc.declare_dram_parameter(
        "smapT", [128, NT * RPAD], dt.int16, isOutput=False
    )
    # all weights in one tensor/DMA: wy cols 0:96, wxh[t][k] at 96+65*(4t+k),
    # wxl (if used) at 876+65*(4t+k)
    WTOT = 96 + 780 + (780 if use_xlo else 0)
    WPACK = nc.declare_dram_parameter(
        "wpack", [128, WTOT], dt.bfloat16, isOutput=False
    )
    OUT = nc.declare_dram_parameter("outbuf", [3, 65, 512], dt.float32, isOutput=True)

    merge_lo = use_xlo or use_ylo
    RW = 30 if use_ylo else 15  # stage-B rhs width: lo cols are zero w/o ylo
    NCH = [sum(len(GROUPS[g]) for g in gc) for gc in GCHUNKS]  # JL per chunk

    with ExitStack() as ctx:
        tc = ctx.enter_context(TileContext(nc))
        # ---- pools ------------------------------------------------------
        pp = ctx.enter_context(tc.tile_pool(name="persist", bufs=1))
        mp = ctx.enter_context(tc.tile_pool(name="masks", bufs=bufs_m))
        if use_xlo:
            bufs_pa = min(bufs_pa, 2)  # psA is 2 banks now; keep total <= 8
        pA = ctx.enter_context(tc.tile_pool(name="psA", bufs=bufs_pa, space="PSUM"))
        pB = ctx.enter_context(tc.tile_pool(name="psB", bufs=2, space="PSUM"))
        pB2 = (
            ctx.enter_context(tc.tile_pool(name="psB2", bufs=2, space="PSUM"))
            if use_xlo
            else None
        )

        # ---- persistent SBUF tensors ------------------------------------
        wpack = pp.tile([128, WTOT], dt.bfloat16, name="wpack", tag="wpack")
        wy = wpack[:, 0:96]
        wxh = [
            [wpack[:, 96 + 65 * (4 * t + k) : 96 + 65 * (4 * t + k) + 65] for k in range(4)]
            for t in range(3)
        ]
        wxl = (
            [
                [
                    wpack[:, 876 + 65 * (4 * t + k) : 876 + 65 * (4 * t + k) + 65]
                    for k in range(4)
                ]
                for t in range(3)
            ]
            if use_xlo
            else None
        )
        AD = [
            pp.tile([96, 6 * IPAD], dt.bfloat16, name=f"ad_{i}", tag=f"ad_{i}")
            for i in range(NAD)
        ]
        # per-chunk transposed A: col(JL, k, q) = 384*JL + 96*k + q
        ATC = [
            pp.tile([128, 384 * 12], dt.bfloat16, name=f"atc_{cc}", tag=f"atc_{cc}")
            for cc in range(len(GCHUNKS))
        ]
        stbig = pp.tile([128, NT * RPAD], dt.int16, name="stbig", tag="stbig")
        outp = [
            pp.tile([65, 512], dt.float32, name=f"outp_{t}", tag=f"outp_{t}")
            for t in range(3)
        ]

        # ---- preload weights: one DMA -----------------------------------
        nc.sync.dma_start(out=wpack[:, :], in_=WPACK[:, :])

        # ---- phase 1: graded input DMAs (small first so masks start early)
        sizes = [int(x) for x in _os.environ.get("KERNEL_GRADE", "2,4,6").split(",")]
        while sum(sizes) < NT:
            sizes.append(min(8, NT - sum(sizes)))
        pos = 0
        for sz in sizes:
            c0 = pos * RPAD
            c1 = (pos + sz) * RPAD
            nc.sync.dma_start(out=stbig[:, c0:c1], in_=smapT[:, c0:c1])
            pos += sz
        drain_i = 0
        for g, Ts in enumerate(GROUPS):
            ad = AD[g % NAD]
            cc = next(i for i, gc in enumerate(GCHUNKS) if g in gc)
            gl = GCHUNKS[cc].index(g)
            for pr in range(0, len(Ts), 2):
                subs = [s for s in (0, 1) if pr + s < len(Ts)]
                psA = pA.tile([96, 1024], dt.float32)  # two tiles, bank-aligned
                for s in subs:
                    jl = pr + s
                    T = Ts[jl]
                    st16 = stbig[:, RPAD * T : RPAD * T + RPAD]
                    masks = []
                    for t in range(3):
                        m = mp.tile([128, RPAD], dt.bfloat16, tag=f"m{t}")
                        use_dve = t < nmask_dve or (t == nmask_dve and (T % 2) == 0)
                        eng = nc.vector if use_dve else nc.gpsimd
                        eng.tensor_scalar(
                            m[:, :],
                            st16,
                            float(t + 1),
                            None,
                            mybir.AluOpType.is_equal,
                        )
                        masks.append(m)
                    for t in range(3):
                        nc.tensor.matmul(
                            psA[32 * t : 32 * t + 32, 512 * s : 512 * s + 512],
                            lhsT=wy[:, 32 * t : 32 * t + 32],
                            rhs=masks[t][:, :],
                            start=True,
                            stop=True,
                        )
                w = 512 * len(subs)
                if (drain_i % 3) < ndrain_dve:
                    nc.vector.tensor_copy(
                        out=ad[0:96, IPAD * pr : IPAD * pr + w], in_=psA[:, 0:w]
                    )
                else:
                    nc.scalar.copy(ad[0:96, IPAD * pr : IPAD * pr + w], psA[:, 0:w])
                drain_i += 1
            # one xbar transpose per group (variable tile count)
            nT = len(Ts)
            jlo = GOFF[g]
            nc.sync.dma_start_transpose(
                ATC[cc][:, 384 * jlo : 384 * jlo + 384 * nT].rearrange(
                    "p (c q) -> p c q", q=96
                ),
                ad[0:96, 0 : nT * IPAD],
            )

        # ---- phase 2: x-reduce + assembly (chunk-major: the last chunk
        # depends on the last transpose, so it must come last on the PE) ----
        for cc, gc in enumerate(GCHUNKS):
            for t in range(3):
                njl = NCH[cc]
                psB = pB.tile([65, 512], dt.float32)
                for k in range(4):
                    rhs = ATC[cc][:, 0 : 384 * njl].rearrange(
                        "p (jl four q) -> p jl four q", four=4, q=96
                    )[:, :, k, 32 * t : 32 * t + RW]
                    nc.tensor.matmul(
                        psB[:, 0 : RW * njl],
                        lhsT=wxh[t][k],
                        rhs=rhs,
                        start=(k == 0),
                        stop=(k == 3),
                    )
                psB2 = None
                if use_xlo:
                    psB2 = pB2.tile([65, 512], dt.float32)
                    for k in range(4):
                        rhs = ATC[cc][:, 0 : 384 * njl].rearrange(
                            "p (jl four q) -> p jl four q", four=4, q=96
                        )[:, :, k, 32 * t : 32 * t + 15]
                        nc.tensor.matmul(
                            psB2[:, 0 : 15 * njl],
                            lhsT=wxl[t][k],
                            rhs=rhs,
                            start=(k == 0),
                            stop=(k == 3),
                        )
                reg = psB[:, 0 : RW * njl].rearrange("p (jl c) -> p jl c", c=RW)
                if use_ylo:
                    nc.vector.scalar_tensor_tensor(
                        out=reg[:, :, 0:15],
                        in0=reg[:, :, 0:15],
                        scalar=1.0,
                        in1=reg[:, :, 15:30],
                        op0=mybir.AluOpType.mult,
                        op1=mybir.AluOpType.add,
                    )
                if use_xlo:
                    nc.vector.scalar_tensor_tensor(
                        out=reg[:, :, 0:15],
                        in0=reg[:, :, 0:15],
                        scalar=1.0,
                        in1=psB2[:, 0 : 15 * njl].rearrange(
                            "p (jl c) -> p jl c", c=15
                        ),
                        op0=mybir.AluOpType.mult,
                        op1=mybir.AluOpType.add,
                    )
                # out[:, 15*T + q] = 64 - psB[JL, q<15]
                T0 = GROUPS[gc[0]][0]
                nfull = njl if cc < 2 else njl - 1
                nc.scalar.activation(
                    outp[t][:, 15 * T0 : 15 * T0 + 15 * nfull],
                    reg[:, 0:nfull, 0:15],
                    mybir.ActivationFunctionType.Copy,
                    bias=64.0,
                    scale=-1.0,
                )
                if cc == 2:  # T=34 tail: only bins 510, 511
                    nc.scalar.activation(
                        outp[t][:, 510:512],
                        reg[:, njl - 1, 0:2],
                        mybir.ActivationFunctionType.Copy,
                        bias=64.0,
                        scale=-1.0,
                    )
                    nc.sync.dma_start(out=OUT[t, :, :], in_=outp[t][:, :])
    return nc


def _get_program(use_xlo, use_ylo):
    key = (use_xlo, use_ylo)
    if key not in _PROG_CACHE:
        _PROG_CACHE[key] = _build_program(use_xlo, use_ylo)
    return _PROG_CACHE[key]


def kernel(site_type_map, site_size_x, site_size_y):
    from concourse.bass_utils import run_bass_kernel_spmd

    smap = np.asarray(site_type_map, dtype=np.int32)
    sx = np.asarray(site_size_x, dtype=np.float32)
    sy = np.asarray(site_size_y, dtype=np.float32)

    WYf = _build_wy(sy)  # [128, 3, 15]
    WXf = _build_wx(sx)  # [IPAD, 3, 65]
    wy_hi, wy_lo = _hi_lo(WYf)
    wx_hi, wx_lo = _hi_lo(WXf)
    use_ylo = bool(np.any(wy_lo.astype(np.float32) != 0))
    use_xlo = bool(np.any(wx_lo.astype(np.float32) != 0))

    # WY device layout [128, 96]: per type 15 hi, 15 lo, 2 zero pad cols
    WTOT = 96 + 780 + (780 if use_xlo else 0)
    wpk = np.zeros((128, WTOT), BF16)
    for t in range(3):
        wpk[:, 32 * t : 32 * t + 15] = wy_hi[:, t, :]
        wpk[:, 32 * t + 15 : 32 * t + 30] = wy_lo[:, t, :]
        for k in range(4):
            o = 96 + 65 * (4 * t + k)
            wpk[:, o : o + 65] = wx_hi[128 * k : 128 * k + 128, t, :]
            if use_xlo:
                o2 = 876 + 65 * (4 * t + k)
                wpk[:, o2 : o2 + 65] = wx_lo[128 * k : 128 * k + 128, t, :]

    nc = _get_program(use_xlo, use_ylo)

    in_maps = []
    for c in range(NCORES):
        sjp = np.zeros((JPAD, RPAD), np.int16)
        sjp[8 : 8 + 4096, :] = smap[512 * c : 512 * c + 512].T
        big = np.empty((128, NT * RPAD), np.int16)
        for T in range(NT):
            big[:, RPAD * T : RPAD * T + RPAD] = sjp[120 * T : 120 * T + 128, :]
        m = {"smapT": big, "wpack": wpk}
        in_maps.append(m)

    res = run_bass_kernel_spmd(
        nc,
        in_maps,
        core_ids=list(range(NCORES)),
        trace=bool(int(os.environ.get("KERNEL_TRACE", "0"))),
    )
    kernel._last_results = res

    # device returns 3 type planes; comp2site=(1,1,2,3) duplicates plane 0
    full = np.empty((4, 512, 512), np.float32)
    for c in range(NCORES):
        ob = res.results[c]["outbuf"]
        full[1:4, 64 * c : 64 * c + 64, :] = ob[:, 0:64, :]
    for c in range(NCORES - 1):
        # spill row: (64-h0) + (64-h1) - 64 = 64 - h0 - h1
        full[1:4, 64 * (c + 1), :] += res.results[c]["outbuf"][:, 64, :] - np.float32(
            64.0
        )
    full[0] = full[1]
    return full



# revision 3
# speedup vs baseline: 1.3844x; 1.3844x over previous
"""Trainium2 Bass kernel for nn_DemandMap (histogram_binning).

Fast path (valid when all site_size_x <= 1, so the x-overlap weight is a
per-type constant and no site crosses an x-bin):

The per-type histogram hist[t, bx, by] = sx_t * sum_j wy_t(j) * C_t(j, bx)
where C_t(j, bx) = |{i in bin bx : type(i,j) == t}|.  The three counts are
computed WITHOUT per-site masks: the host relabels each site to
mu = {0:0, 1:1, 2:16, 3:256, 4:0}[type] in fp8-e5m2 (exact powers of two)
and the PE contracts mu over x with an all-ones banded matrix:

  S(j, bx) = sum_{i in bin bx} mu(i, j) = C1 + 16*C2 + 256*C3

Since C1+C2+C3 <= 8 per (j, bx), S <= 2048 and every digit is recoverable:
S is drained to int16, shifted (>>4, >>8) and cast to fp16 (all values
<= 2048, fp16-exact).  The base-16 digit recombination is folded into the
second (y-reduce) matmul weights:

  hist rows = WA^T D0 + WB^T D4 + WC^T D8,   D0 = S, D4 = S>>4, D8 = S>>8
  WA = [WY1 |  0  | 0]   WB = [-16*WY1 | WY2 | 0]   WC = [0 | -16*WY2 | WY3]

Device dataflow per core (partitions = x-sites, free axis = y-sites):
  mu tiles [128 i, 4224 jp] x 4 i-blocks; j-tiles of 128 rows at stride 120
  (8-row halo) exactly like the classic layout, but sliced on the FREE axis
  so no halo duplication is materialized.
  stage 1: matmul(psS[128 j, 16], lhsT=mu[:, 120T:120T+128], rhs=WX8) per
    (tile, i-block) -- output width 16, data as the stationary operand.
  drain+unpack per 7-tile group: ACT copy psum->int16, DVE shifts, casted
    fp16 copies spread over DVE/ACT/GPSIMD.
  stage 2: 3 (or 6 with fp16-lo weights) accumulating matmuls per tile,
    output [45, 64] into a persistent psC [45, 2240] PSUM tensor.
  output: DMA straight from PSUM to DRAM per group; host reassembles and
  computes 64 - hist, duplicating plane 0 (comp2site = (1,1,2,3)).

A general fallback (any site_size_x) keeps the previous mask-based kernel.
"""

import json
import os

import numpy as np
import ml_dtypes

BF16 = ml_dtypes.bfloat16
FP8E5 = ml_dtypes.float8_e5m2

NCORES = 8

# ---- fast-path geometry ----
NBLK = 4          # 128-row x-blocks per core
JP = 4224         # 8 zero pad + 4096 + 120 tail pad (y axis, free)
NT = 35           # j-tiles, stride 120, each owns 15 y-bins
GT = 7            # tiles per drain group
NG = 5            # drain groups
MUW = NBLK * JP   # mu sbuf/dram cols

# ---- general-path geometry (fallback kernel, unchanged) ----
RPAD = 512
IPAD = 512
JPAD = 4224
GROUPS = [list(range(6 * g, 6 * g + 6)) for g in range(4)] + [
    [24, 25, 26, 27, 28],
    [29, 30, 31],
    [32, 33, 34],
]
GCHUNKS = [[0, 1], [2, 3], [4, 5, 6]]
GOFF = {0: 0, 1: 6, 2: 0, 3: 6, 4: 0, 5: 5, 6: 8}
NAD = int(os.environ.get("KERNEL_NAD", "4"))

_PROG_CACHE = {}
_WSPLIT_DONE = [False]


def _install_wait_split():
    """This walrus build accepts only ONE sync wait per instruction; Tile
    attaches N.  Rewrite the BIR JSON: hoist all-but-one wait onto fresh
    same-engine EventSemaphore waits inserted before the offender."""
    if _WSPLIT_DONE[0]:
        return
    import concourse.bass as bass

    orig = bass.Bass.to_json_bytes

    def split(self, *a, **k):
        m = json.loads(orig(self, *a, **k))
        n = [0]
        for fn in m["functions"]:
            for blk in fn["blocks"]:
                out = []
                for ins in blk["instructions"]:
                    si = ins.get("sync_info")
                    waits = si.get("on_wait") if si else None
                    if waits and len(waits) > 1:
                        for w in waits[:-1]:
                            n[0] += 1
                            nop = {
                                "engine": ins["engine"],
                                "ins": [],
                                "outs": [],
                                "name": f"WSPLIT-{n[0]}",
                                "opcode": "EventSemaphore",
                                "sync_info": {"on_update": [], "on_wait": [w]},
                            }
                            if "debug" in ins:
                                nop["debug"] = ins["debug"]
                            out.append(nop)
                        si["on_wait"] = [waits[-1]]
                    out.append(ins)
                blk["instructions"] = out
        return json.dumps(m).encode()

    bass.Bass.to_json_bytes = split
    _WSPLIT_DONE[0] = True


def _oxy_weights(size_f32):
    """Per-residue overlap weights, matching the reference f32 formulas."""
    r = np.arange(8, dtype=np.float32)
    o0 = np.maximum(np.minimum(np.float32(8.0) - r, size_f32), np.float32(0.0))
    o1 = np.maximum(
        np.minimum(np.float32(16.0) - r, size_f32) - (np.float32(8.0) - r),
        np.float32(0.0),
    )
    return o0.astype(np.float32), o1.astype(np.float32)


def _build_wy(sy):
    """f32 [128, 3, 15]: y-stage stationary (y-halo tile form), types 1..3.
    Tile partition p holds padded j = 120T + p (jp = j + 8), so ry = p % 8;
    bin q gets ky=0 rows p//8 == q+1 and ky=1 rows p//8 == q."""
    W = np.zeros((128, 3, 15), np.float32)
    for tp in range(3):
        o0, o1 = _oxy_weights(np.float32(sy[tp + 1]))
        for p in range(128):
            if 0 <= p // 8 - 1 < 15:
                W[p, tp, p // 8 - 1] += o0[p % 8]
            if p // 8 < 15:
                W[p, tp, p // 8] += o1[p % 8]
    return W


# ======================================================================
# fast path
# ======================================================================

def _build_program_fast(use_lo):
    _install_wait_split()
    import concourse.bass as bass
    import concourse.mybir as mybir
    from concourse.tile import TileContext
    from contextlib import ExitStack

    dt = mybir.dt
    nc = bass.Bass()
    MUT = nc.declare_dram_parameter("muT", [128, MUW], dt.float8e5, isOutput=False)
    WX8 = nc.declare_dram_parameter("wx8", [128, 16], dt.float8e5, isOutput=False)
    NW = 288 if use_lo else 144  # WA|WB|WC hi (3*48) + optional lo
    WST = nc.declare_dram_parameter("wst", [128, NW], dt.float16, isOutput=False)
    OUT = nc.declare_dram_parameter("outbuf", [45, NT * 64], dt.float32, isOutput=True)

    with ExitStack() as ctx:
        tc = ctx.enter_context(TileContext(nc))
        pp = ctx.enter_context(tc.tile_pool(name="persist", bufs=1))
        pS = ctx.enter_context(tc.tile_pool(name="psS", bufs=2, space="PSUM"))
        pC = ctx.enter_context(tc.tile_pool(name="psC", bufs=1, space="PSUM"))

        mu = pp.tile([128, MUW], dt.float8e5, name="mu", tag="mu")
        wx8 = pp.tile([128, 16], dt.float8e5, name="wx8", tag="wx8")
        wst = pp.tile([128, NW], dt.float16, name="wst", tag="wst")
        u16 = pp.tile([128, NT * 64], dt.int16, name="u16", tag="u16")
        s4 = pp.tile([128, NT * 64], dt.int16, name="s4", tag="s4")
        s8 = pp.tile([128, NT * 64], dt.int16, name="s8", tag="s8")
        D0 = pp.tile([128, NT * 64], dt.float16, name="d0", tag="d0")
        D4 = pp.tile([128, NT * 64], dt.float16, name="d4", tag="d4")
        D8 = pp.tile([128, NT * 64], dt.float16, name="d8", tag="d8")
        outp = pp.tile([45, NT * 64], dt.float32, name="outp", tag="outp")
        psC = pC.tile([45, NT * 64], dt.float32, name="psc")

        nc.sync.dma_start(out=wx8[:, :], in_=WX8[:, :])
        nc.sync.dma_start(out=wst[:, :], in_=WST[:, :])

        # input chunks aligned to drain groups: group g = tiles 7g..7g+7,
        # jp range [840g, 840g+848) (+tail pad on the last)
        mu_dram = MUT[:, :].rearrange("p (b j) -> p b j", b=NBLK)
        mu_sb = mu[:, :].rearrange("p (b j) -> p b j", b=NBLK)
        for g in range(NG):
            j0 = 840 * g
            j1 = JP if g == NG - 1 else 840 * g + 848
            nc.sync.dma_start(out=mu_sb[:, :, j0:j1], in_=mu_dram[:, :, j0:j1])

        for g in range(NG):
            psS = pS.tile([128, GT * 64], dt.float32)
            for tau in range(GT):
                T = GT * g + tau
                for b in range(NBLK):
                    nc.tensor.matmul(
                        psS[:, 64 * tau + 16 * b : 64 * tau + 16 * b + 16],
                        lhsT=mu[:, JP * b + 120 * T : JP * b + 120 * T + 128],
                        rhs=wx8[:, :],
                        start=True,
                        stop=True,
                    )
            cg = slice(GT * 64 * g, GT * 64 * (g + 1))
            # drain S -> int16 (ACT), then unpack digits
            nc.scalar.copy(u16[:, cg], psS[:, :])
            nc.vector.tensor_scalar(
                s4[:, cg], u16[:, cg], 4, None, mybir.AluOpType.logical_shift_right
            )
            nc.vector.tensor_scalar(
                s8[:, cg], u16[:, cg], 8, None, mybir.AluOpType.logical_shift_right
            )
            nc.vector.tensor_copy(out=D0[:, cg], in_=u16[:, cg])
            nc.gpsimd.tensor_copy(out=D4[:, cg], in_=s4[:, cg])
            nc.scalar.copy(D8[:, cg], s8[:, cg])
            for tau in range(GT):
                T = GT * g + tau
                ct = slice(64 * T, 64 * T + 64)
                nmm = 6 if use_lo else 3
                srcs = [(0, D0), (48, D4), (96, D8)]
                if use_lo:
                    srcs += [(144, D0), (192, D4), (240, D8)]
                for k, (wc, DD) in enumerate(srcs):
                    nc.tensor.matmul(
                        psC[:, ct],
                        lhsT=wst[:, wc : wc + 45],
                        rhs=DD[:, ct],
                        start=(k == 0),
                        stop=(k == nmm - 1),
                    )
            nc.scalar.activation(
                outp[:, cg],
                psC[:, cg],
                mybir.ActivationFunctionType.Copy,
                bias=64.0,
                scale=-1.0,
            )
            nc.sync.dma_start(out=OUT[:, cg], in_=outp[:, cg])
    return nc


def _pack_mu(smap_core):
    """[512, 4096] int -> [128, MUW] fp8e5 mu tiles (jp-padded, block-major)."""
    lut = np.zeros(5, np.float32)
    lut[1], lut[2], lut[3] = 1.0, 16.0, 256.0
    mu = lut[smap_core]  # [512 i, 4096 j] f32
    out = np.zeros((128, MUW), FP8E5)
    m8 = mu.astype(FP8E5)
    for b in range(NBLK):
        out[:, JP * b + 8 : JP * b + 8 + 4096] = m8[128 * b : 128 * b + 128, :]
    return out


def _fast_weights(sx, sy):
    """Stage-2 fp16 weight pack [128, 144] (+144 lo): WA|WB|WC at 48-col
    offsets, 45 used cols each (3 types x 15 bins on the output partition
    axis: rows 15t+q)."""
    WYf = _build_wy(sy)  # [128, 3, 15] f32
    for t in range(3):
        WYf[:, t, :] *= np.float32(max(min(float(sx[t + 1]), 1.0), 0.0))
    WA = np.zeros((128, 45), np.float32)
    WB = np.zeros((128, 45), np.float32)
    WC = np.zeros((128, 45), np.float32)
    WA[:, 0:15] = WYf[:, 0]
    WB[:, 0:15] = -16.0 * WYf[:, 0]
    WB[:, 15:30] = WYf[:, 1]
    WC[:, 15:30] = -16.0 * WYf[:, 1]
    WC[:, 30:45] = WYf[:, 2]
    hi = np.zeros((128, 144), np.float32)
    lo = np.zeros((128, 144), np.float32)
    for o, W in ((0, WA), (48, WB), (96, WC)):
        h = W.astype(np.float16).astype(np.float32)
        hi[:, o : o + 45] = h
        lo[:, o : o + 45] = W - h
    use_lo = bool(np.any(lo != 0))
    if use_lo:
        pack = np.concatenate([hi, lo], axis=1).astype(np.float16)
    else:
        pack = hi.astype(np.float16)
    return pack, use_lo


def _kernel_fast(smap, sx, sy):
    from concourse.bass_utils import run_bass_kernel_spmd

    wst, use_lo = _fast_weights(sx, sy)
    wx8 = np.zeros((128, 16), np.float32)
    for i in range(128):
        wx8[i, i // 8] = 1.0
    wx8 = wx8.astype(FP8E5)

    key = ("fast", use_lo)
    if key not in _PROG_CACHE:
        _PROG_CACHE[key] = _build_program_fast(use_lo)
    nc = _PROG_CACHE[key]

    in_maps = []
    for c in range(NCORES):
        in_maps.append(
            {
                "muT": _pack_mu(smap[512 * c : 512 * c + 512]),
                "wx8": wx8,
                "wst": wst,
            }
        )

    res = run_bass_kernel_spmd(
        nc,
        in_maps,
        core_ids=list(range(NCORES)),
        trace=bool(int(os.environ.get("KERNEL_TRACE", "0"))),
    )
    kernel._last_results = res

    full = np.empty((4, 512, 512), np.float32)
    for c in range(NCORES):
        ob = res.results[c]["outbuf"]  # [45, 35*64]
        # rows 15t+q, cols 64T+bx -> hist[t, bx, 15T+q]
        h = ob.reshape(3, 15, NT, 64).transpose(0, 3, 2, 1).reshape(3, 64, NT * 15)
        full[1:4, 64 * c : 64 * c + 64, :] = h[:, :, :512]
    full[0] = full[1]
    return full


# ======================================================================
# general fallback (previous mask-based kernel, unchanged)
# ======================================================================

def _nbins(T):
    return 15 if T < 34 else 2


def _build_wx(sx):
    """f32 [IPAD, 3, 65]: x-stage stationary.  Local row r = i - 512c; col 64
    is the spill bin (kx=1 of the last 8 rows -> next core's first bin)."""
    W = np.zeros((IPAD, 3, 65), np.float32)
    for tp in range(3):
        o0, o1 = _oxy_weights(np.float32(sx[tp + 1]))
        for r in range(512):
            W[r, tp, r // 8] += o0[r % 8]
            W[r, tp, r // 8 + 1] += o1[r % 8]
    return W


def _hi_lo(w):
    hi = w.astype(BF16)
    lo = (w - hi.astype(np.float32)).astype(BF16)
    return hi, lo


def _build_program(use_xlo, use_ylo):
    _install_wait_split()
    import os as _os
    import concourse.bass as bass
    import concourse.mybir as mybir
    from concourse.tile import TileContext
    from contextlib import ExitStack

    bufs_m = int(_os.environ.get("KERNEL_BUFS_M", "4"))
    bufs_pa = int(_os.environ.get("KERNEL_BUFS_PA", "3"))
    nmask_dve = int(_os.environ.get("KERNEL_MASK_DVE", "2"))
    ndrain_dve = int(_os.environ.get("KERNEL_DRAIN_DVE", "0"))

    dt = mybir.dt
    nc = bass.Bass()
    smapT = nc.declare_dram_parameter(
        "smapT", [128, NT * RPAD], dt.int16, isOutput=False
    )
    WTOT = 96 + 780 + (780 if use_xlo else 0)
    WPACK = nc.declare_dram_parameter(
        "wpack", [128, WTOT], dt.bfloat16, isOutput=False
    )
    OUT = nc.declare_dram_parameter("outbuf", [3, 65, 512], dt.float32, isOutput=True)

    RW = 30 if use_ylo else 15
    NCH = [sum(len(GROUPS[g]) for g in gc) for gc in GCHUNKS]

    with ExitStack() as ctx:
        tc = ctx.enter_context(TileContext(nc))
        pp = ctx.enter_context(tc.tile_pool(name="persist", bufs=1))
        mp = ctx.enter_context(tc.tile_pool(name="masks", bufs=bufs_m))
        if use_xlo:
            bufs_pa = min(bufs_pa, 2)
        pA = ctx.enter_context(tc.tile_pool(name="psA", bufs=bufs_pa, space="PSUM"))
        pB = ctx.enter_context(tc.tile_pool(name="psB", bufs=2, space="PSUM"))
        pB2 = (
            ctx.enter_context(tc.tile_pool(name="psB2", bufs=2, space="PSUM"))
            if use_xlo
            else None
        )

        wpack = pp.tile([128, WTOT], dt.bfloat16, name="wpack", tag="wpack")
        wy = wpack[:, 0:96]
        wxh = [
            [wpack[:, 96 + 65 * (4 * t + k) : 96 + 65 * (4 * t + k) + 65] for k in range(4)]
            for t in range(3)
        ]
        wxl = (
            [
                [
                    wpack[:, 876 + 65 * (4 * t + k) : 876 + 65 * (4 * t + k) + 65]
                    for k in range(4)
                ]
                for t in range(3)
            ]
            if use_xlo
            else None
        )
        AD = [
            pp.tile([96, 6 * IPAD], dt.bfloat16, name=f"ad_{i}", tag=f"ad_{i}")
            for i in range(NAD)
        ]
        ATC = [
            pp.tile([128, 384 * 12], dt.bfloat16, name=f"atc_{cc}", tag=f"atc_{cc}")
            for cc in range(len(GCHUNKS))
        ]
        stbig = pp.tile([128, NT * RPAD], dt.int16, name="stbig", tag="stbig")
        outp = [
            pp.tile([65, 512], dt.float32, name=f"outp_{t}", tag=f"outp_{t}")
            for t in range(3)
        ]

        nc.sync.dma_start(out=wpack[:, :], in_=WPACK[:, :])

        sizes = [int(x) for x in _os.environ.get("KERNEL_GRADE", "2,4,6").split(",")]
        while sum(sizes) < NT:
            sizes.append(min(8, NT - sum(sizes)))
        pos = 0
        for sz in sizes:
            c0 = pos * RPAD
            c1 = (pos + sz) * RPAD
            nc.sync.dma_start(out=stbig[:, c0:c1], in_=smapT[:, c0:c1])
            pos += sz
        drain_i = 0
        for g, Ts in enumerate(GROUPS):
            ad = AD[g % NAD]
            cc = next(i for i, gc in enumerate(GCHUNKS) if g in gc)
            for pr in range(0, len(Ts), 2):
                subs = [s for s in (0, 1) if pr + s < len(Ts)]
                psA = pA.tile([96, 1024], dt.float32)
                for s in subs:
                    jl = pr + s
                    T = Ts[jl]
                    st16 = stbig[:, RPAD * T : RPAD * T + RPAD]
                    masks = []
                    for t in range(3):
                        m = mp.tile([128, RPAD], dt.bfloat16, tag=f"m{t}")
                        use_dve = t < nmask_dve or (t == nmask_dve and (T % 2) == 0)
                        eng = nc.vector if use_dve else nc.gpsimd
                        eng.tensor_scalar(
                            m[:, :],
                            st16,
                            float(t + 1),
                            None,
                            mybir.AluOpType.is_equal,
                        )
                        masks.append(m)
                    for t in range(3):
                        nc.tensor.matmul(
                            psA[32 * t : 32 * t + 32, 512 * s : 512 * s + 512],
                            lhsT=wy[:, 32 * t : 32 * t + 32],
                            rhs=masks[t][:, :],
                            start=True,
                            stop=True,
                        )
                w = 512 * len(subs)
                if (drain_i % 3) < ndrain_dve:
                    nc.vector.tensor_copy(
                        out=ad[0:96, IPAD * pr : IPAD * pr + w], in_=psA[:, 0:w]
                    )
                else:
                    nc.scalar.copy(ad[0:96, IPAD * pr : IPAD * pr + w], psA[:, 0:w])
                drain_i += 1
            nT = len(Ts)
            jlo = GOFF[g]
            nc.sync.dma_start_transpose(
                ATC[cc][:, 384 * jlo : 384 * jlo + 384 * nT].rearrange(
                    "p (c q) -> p c q", q=96
                ),
                ad[0:96, 0 : nT * IPAD],
            )

        for cc, gc in enumerate(GCHUNKS):
            for t in range(3):
                njl = NCH[cc]
                psB = pB.tile([65, 512], dt.float32)
                for k in range(4):
                    rhs = ATC[cc][:, 0 : 384 * njl].rearrange(
                        "p (jl four q) -> p jl four q", four=4, q=96
                    )[:, :, k, 32 * t : 32 * t + RW]
                    nc.tensor.matmul(
                        psB[:, 0 : RW * njl],
                        lhsT=wxh[t][k],
                        rhs=rhs,
                        start=(k == 0),
                        stop=(k == 3),
                    )
                psB2 = None
                if use_xlo:
                    psB2 = pB2.tile([65, 512], dt.float32)
                    for k in range(4):
                        rhs = ATC[cc][:, 0 : 384 * njl].rearrange(
                            "p (jl four q) -> p jl four q", four=4, q=96
                        )[:, :, k, 32 * t : 32 * t + 15]
                        nc.tensor.matmul(
                            psB2[:, 0 : 15 * njl],
                            lhsT=wxl[t][k],
                            rhs=rhs,
                            start=(k == 0),
                            stop=(k == 3),
                        )
                reg = psB[:, 0 : RW * njl].rearrange("p (jl c) -> p jl c", c=RW)
                if use_ylo:
                    nc.vector.scalar_tensor_tensor(
                        out=reg[:, :, 0:15],
                        in0=reg[:, :, 0:15],
                        scalar=1.0,
                        in1=reg[:, :, 15:30],
                        op0=mybir.AluOpType.mult,
                        op1=mybir.AluOpType.add,
                    )
                if use_xlo:
                    nc.vector.scalar_tensor_tensor(
                        out=reg[:, :, 0:15],
                        in0=reg[:, :, 0:15],
                        scalar=1.0,
                        in1=psB2[:, 0 : 15 * njl].rearrange(
                            "p (jl c) -> p jl c", c=15
                        ),
                        op0=mybir.AluOpType.mult,
                        op1=mybir.AluOpType.add,
                    )
                T0 = GROUPS[gc[0]][0]
                nfull = njl if cc < 2 else njl - 1
                nc.scalar.activation(
                    outp[t][:, 15 * T0 : 15 * T0 + 15 * nfull],
                    reg[:, 0:nfull, 0:15],
                    mybir.ActivationFunctionType.Copy,
                    bias=64.0,
                    scale=-1.0,
                )
                if cc == 2:
                    nc.scalar.activation(
                        outp[t][:, 510:512],
                        reg[:, njl - 1, 0:2],
                        mybir.ActivationFunctionType.Copy,
                        bias=64.0,
                        scale=-1.0,
                    )
                    nc.sync.dma_start(out=OUT[t, :, :], in_=outp[t][:, :])
    return nc


def _kernel_general(smap, sx, sy):
    from concourse.bass_utils import run_bass_kernel_spmd

    WYf = _build_wy(sy)
    WXf = _build_wx(sx)
    wy_hi, wy_lo = _hi_lo(WYf)
    wx_hi, wx_lo = _hi_lo(WXf)
    use_ylo = bool(np.any(wy_lo.astype(np.float32) != 0))
    use_xlo = bool(np.any(wx_lo.astype(np.float32) != 0))

    WTOT = 96 + 780 + (780 if use_xlo else 0)
    wpk = np.zeros((128, WTOT), BF16)
    for t in range(3):
        wpk[:, 32 * t : 32 * t + 15] = wy_hi[:, t, :]
        wpk[:, 32 * t + 15 : 32 * t + 30] = wy_lo[:, t, :]
        for k in range(4):
            o = 96 + 65 * (4 * t + k)
            wpk[:, o : o + 65] = wx_hi[128 * k : 128 * k + 128, t, :]
            if use_xlo:
                o2 = 876 + 65 * (4 * t + k)
                wpk[:, o2 : o2 + 65] = wx_lo[128 * k : 128 * k + 128, t, :]

    key = (use_xlo, use_ylo)
    if key not in _PROG_CACHE:
        _PROG_CACHE[key] = _build_program(use_xlo, use_ylo)
    nc = _PROG_CACHE[key]

    in_maps = []
    for c in range(NCORES):
        sjp = np.zeros((JPAD, RPAD), np.int16)
        sjp[8 : 8 + 4096, :] = smap[512 * c : 512 * c + 512].T
        big = np.empty((128, NT * RPAD), np.int16)
        for T in range(NT):
            big[:, RPAD * T : RPAD * T + RPAD] = sjp[120 * T : 120 * T + 128, :]
        in_maps.append({"smapT": big, "wpack": wpk})

    res = run_bass_kernel_spmd(
        nc,
        in_maps,
        core_ids=list(range(NCORES)),
        trace=bool(int(os.environ.get("KERNEL_TRACE", "0"))),
    )
    kernel._last_results = res

    full = np.empty((4, 512, 512), np.float32)
    for c in range(NCORES):
        ob = res.results[c]["outbuf"]
        full[1:4, 64 * c : 64 * c + 64, :] = ob[:, 0:64, :]
    for c in range(NCORES - 1):
        full[1:4, 64 * (c + 1), :] += res.results[c]["outbuf"][:, 64, :] - np.float32(
            64.0
        )
    full[0] = full[1]
    return full


def kernel(site_type_map, site_size_x, site_size_y):
    smap = np.asarray(site_type_map, dtype=np.int32)
    sx = np.asarray(site_size_x, dtype=np.float32)
    sy = np.asarray(site_size_y, dtype=np.float32)

    if bool(np.all(sx[1:4] <= 1.0)):
        return _kernel_fast(smap, sx, sy)
    return _kernel_general(smap, sx, sy)


# revision 4
# speedup vs baseline: 2.1901x; 1.5820x over previous
"""Trainium2 Bass kernel for nn_DemandMap (histogram_binning).

Fast path (valid when all site_size_x <= 1, so the x-overlap weight is a
per-type constant and no site crosses an x-bin):

The per-type histogram hist[t, bx, by] = sx_t * sum_j wy_t(j) * C_t(j, bx)
where C_t(j, bx) = |{i in bin bx : type(i,j) == t}|.  The three counts are
computed WITHOUT per-site masks: the host relabels each site to
mu = {0:0, 1:1, 2:16, 3:256, 4:0}[type] in fp8-e5m2 (exact powers of two)
and the PE contracts mu over x with an all-ones banded matrix:

  S(j, bx) = sum_{i in bin bx} mu(i, j) = C1 + 16*C2 + 256*C3

Since C1+C2+C3 <= 8 per (j, bx), S <= 2048 and every digit is recoverable:
S is drained to int16, shifted (>>4, >>8) and cast to fp16 (all values
<= 2048, fp16-exact).  The base-16 digit recombination is folded into the
second (y-reduce) matmul weights:

  hist rows = WA^T D0 + WB^T D4 + WC^T D8,   D0 = S, D4 = S>>4, D8 = S>>8
  WA = [WY1 |  0  | 0]   WB = [-16*WY1 | WY2 | 0]   WC = [0 | -16*WY2 | WY3]

Device dataflow per core (partitions = x-sites, free axis = y-sites):
  mu tiles [128 i, 4224 jp] x 4 i-blocks; j-tiles of 128 rows at stride 120
  (8-row halo) exactly like the classic layout, but sliced on the FREE axis
  so no halo duplication is materialized.
  stage 1: matmul(psS[128 j, 16], lhsT=mu[:, 120T:120T+128], rhs=WX8) per
    (tile, i-block) -- output width 16, data as the stationary operand.
  drain+unpack per 7-tile group: ACT copy psum->int16, DVE shifts, casted
    fp16 copies spread over DVE/ACT/GPSIMD.
  stage 2: 3 (or 6 with fp16-lo weights) accumulating matmuls per tile,
    output [45, 64] into a persistent psC [45, 2240] PSUM tensor.
  output: DMA straight from PSUM to DRAM per group; host reassembles and
  computes 64 - hist, duplicating plane 0 (comp2site = (1,1,2,3)).

A general fallback (any site_size_x) keeps the previous mask-based kernel.
"""

import json
import os

import numpy as np
import ml_dtypes

BF16 = ml_dtypes.bfloat16
FP8E5 = ml_dtypes.float8_e5m2

NCORES = 8

# ---- fast-path geometry ----
NBLK = 4          # 128-row x-blocks per core
JP = 4224         # 8 zero pad + 4096 + 120 tail pad (y axis, free)
NT = 35           # j-tiles, stride 120, each owns 15 y-bins
GT = 7            # tiles per drain group
NG = 5            # drain groups
MUW = NBLK * JP   # mu sbuf/dram cols

# ---- general-path geometry (fallback kernel, unchanged) ----
RPAD = 512
IPAD = 512
JPAD = 4224
GROUPS = [list(range(6 * g, 6 * g + 6)) for g in range(4)] + [
    [24, 25, 26, 27, 28],
    [29, 30, 31],
    [32, 33, 34],
]
GCHUNKS = [[0, 1], [2, 3], [4, 5, 6]]
GOFF = {0: 0, 1: 6, 2: 0, 3: 6, 4: 0, 5: 5, 6: 8}
NAD = int(os.environ.get("KERNEL_NAD", "4"))

_PROG_CACHE = {}
_WSPLIT_DONE = [False]


def _install_wait_split():
    """This walrus build accepts only ONE sync wait per instruction; Tile
    attaches N.  Rewrite the BIR JSON: hoist all-but-one wait onto fresh
    same-engine EventSemaphore waits inserted before the offender."""
    if _WSPLIT_DONE[0]:
        return
    import concourse.bass as bass

    orig = bass.Bass.to_json_bytes

    def split(self, *a, **k):
        m = json.loads(orig(self, *a, **k))
        n = [0]
        for fn in m["functions"]:
            for blk in fn["blocks"]:
                out = []
                for ins in blk["instructions"]:
                    si = ins.get("sync_info")
                    waits = si.get("on_wait") if si else None
                    if waits and len(waits) > 1:
                        for w in waits[:-1]:
                            n[0] += 1
                            nop = {
                                "engine": ins["engine"],
                                "ins": [],
                                "outs": [],
                                "name": f"WSPLIT-{n[0]}",
                                "opcode": "EventSemaphore",
                                "sync_info": {"on_update": [], "on_wait": [w]},
                            }
                            if "debug" in ins:
                                nop["debug"] = ins["debug"]
                            out.append(nop)
                        si["on_wait"] = [waits[-1]]
                    out.append(ins)
                blk["instructions"] = out
        return json.dumps(m).encode()

    bass.Bass.to_json_bytes = split
    _WSPLIT_DONE[0] = True


def _oxy_weights(size_f32):
    """Per-residue overlap weights, matching the reference f32 formulas."""
    r = np.arange(8, dtype=np.float32)
    o0 = np.maximum(np.minimum(np.float32(8.0) - r, size_f32), np.float32(0.0))
    o1 = np.maximum(
        np.minimum(np.float32(16.0) - r, size_f32) - (np.float32(8.0) - r),
        np.float32(0.0),
    )
    return o0.astype(np.float32), o1.astype(np.float32)


def _build_wy(sy):
    """f32 [128, 3, 15]: y-stage stationary (y-halo tile form), types 1..3.
    Tile partition p holds padded j = 120T + p (jp = j + 8), so ry = p % 8;
    bin q gets ky=0 rows p//8 == q+1 and ky=1 rows p//8 == q."""
    W = np.zeros((128, 3, 15), np.float32)
    for tp in range(3):
        o0, o1 = _oxy_weights(np.float32(sy[tp + 1]))
        for p in range(128):
            if 0 <= p // 8 - 1 < 15:
                W[p, tp, p // 8 - 1] += o0[p % 8]
            if p // 8 < 15:
                W[p, tp, p // 8] += o1[p % 8]
    return W


# ======================================================================
# fast path
# ======================================================================

def _build_program_fast(use_lo):
    _install_wait_split()
    import concourse.bass as bass
    import concourse.mybir as mybir
    from concourse.tile import TileContext
    from contextlib import ExitStack

    dt = mybir.dt
    nc = bass.Bass()
    MUT = nc.declare_dram_parameter("muT", [128, MUW], dt.float8e5, isOutput=False)
    WX8 = nc.declare_dram_parameter("wx8", [128, 16], dt.float8e5, isOutput=False)
    NW = 288 if use_lo else 144  # WA|WB|WC hi (3*48) + optional lo
    WST = nc.declare_dram_parameter("wst", [128, NW], dt.float16, isOutput=False)
    OUT = nc.declare_dram_parameter("outbuf", [45, NT * 64], dt.float32, isOutput=True)

    GW = GT * 64  # 448 result cols per group

    with ExitStack() as ctx:
        tc = ctx.enter_context(TileContext(nc))
        pp = ctx.enter_context(tc.tile_pool(name="persist", bufs=1))
        gp = ctx.enter_context(tc.tile_pool(name="groups", bufs=3))
        pS = ctx.enter_context(tc.tile_pool(name="psS", bufs=3, space="PSUM"))
        pC = ctx.enter_context(tc.tile_pool(name="psC", bufs=3, space="PSUM"))

        wx8 = pp.tile([128, 16], dt.float8e5, name="wx8", tag="wx8")
        wst = pp.tile([128, NW], dt.float16, name="wst", tag="wst")
        mus = [
            pp.tile(
                [128, NBLK * (JP - 840 * (NG - 1) if g == NG - 1 else 848)],
                dt.float8e5,
                name=f"mu{g}",
                tag=f"mu{g}",
            )
            for g in range(NG)
        ]

        # weight DMAs ride the ACT queue so the input chunks head the SP queue
        nc.scalar.dma_start(out=wx8[:, :], in_=WX8[:, :])
        nc.scalar.dma_start(out=wst[:, :], in_=WST[:, :])

        # input chunks aligned to drain groups: group g = tiles 7g..7g+7,
        # jp range [840g, 840g+848) (+tail pad on the last)
        mu_dram = MUT[:, :].rearrange("p (b j) -> p b j", b=NBLK)
        for g in range(NG):
            j0 = 840 * g
            j1 = JP if g == NG - 1 else 840 * g + 848
            nc.sync.dma_start(
                out=mus[g][:, :].rearrange("p (b j) -> p b j", b=NBLK),
                in_=mu_dram[:, :, j0:j1],
            )

        for g in range(NG):
            cw = mus[g].shape[1] // NBLK
            psS = pS.tile([128, GW], dt.float32)
            for tau in range(GT):
                T = GT * g + tau
                for b in range(NBLK):
                    nc.tensor.matmul(
                        psS[:, 64 * tau + 16 * b : 64 * tau + 16 * b + 16],
                        lhsT=mus[g][:, cw * b + 120 * tau : cw * b + 120 * tau + 128],
                        rhs=wx8[:, :],
                        start=True,
                        stop=True,
                    )
            # drain S -> int16 (ACT), then unpack digits (DVE), D8 on gpsimd
            u16 = gp.tile([128, GW], dt.int16, tag="u16")
            s4 = gp.tile([128, GW], dt.int16, tag="s4")
            s8 = gp.tile([128, GW], dt.int16, tag="s8")
            D0 = gp.tile([128, GW], dt.float16, tag="d0")
            D4 = gp.tile([128, GW], dt.float16, tag="d4")
            D8 = gp.tile([128, GW], dt.float16, tag="d8")
            nc.scalar.copy(u16[:, :], psS[:, :])
            nc.vector.tensor_scalar(
                s4[:, :], u16[:, :], 4, None, mybir.AluOpType.logical_shift_right
            )
            nc.vector.tensor_scalar(
                s8[:, :], u16[:, :], 8, None, mybir.AluOpType.logical_shift_right
            )
            nc.vector.tensor_copy(out=D0[:, :], in_=u16[:, :])
            nc.vector.tensor_copy(out=D4[:, :], in_=s4[:, :])
            nc.gpsimd.tensor_copy(out=D8[:, :], in_=s8[:, :])
            psC = pC.tile([45, 512], dt.float32)
            for tau in range(GT):
                ct = slice(64 * tau, 64 * tau + 64)
                nmm = 6 if use_lo else 3
                srcs = [(0, D0), (48, D4), (96, D8)]
                if use_lo:
                    srcs += [(144, D0), (192, D4), (240, D8)]
                for k, (wc, DD) in enumerate(srcs):
                    nc.tensor.matmul(
                        psC[:, ct],
                        lhsT=wst[:, wc : wc + 45],
                        rhs=DD[:, ct],
                        start=(k == 0),
                        stop=(k == nmm - 1),
                    )
            outp = gp.tile([45, GW], dt.float32, tag="outp")
            nc.scalar.activation(
                outp[:, :],
                psC[:, 0:GW],
                mybir.ActivationFunctionType.Copy,
                bias=64.0,
                scale=-1.0,
            )
            nc.sync.dma_start(out=OUT[:, GW * g : GW * (g + 1)], in_=outp[:, :])
    return nc


def _pack_mu(smap_core):
    """[512, 4096] int -> [128, MUW] fp8e5 mu tiles (jp-padded, block-major)."""
    lut = np.zeros(5, np.float32)
    lut[1], lut[2], lut[3] = 1.0, 16.0, 256.0
    mu = lut[smap_core]  # [512 i, 4096 j] f32
    out = np.zeros((128, MUW), FP8E5)
    m8 = mu.astype(FP8E5)
    for b in range(NBLK):
        out[:, JP * b + 8 : JP * b + 8 + 4096] = m8[128 * b : 128 * b + 128, :]
    return out


def _fast_weights(sx, sy):
    """Stage-2 fp16 weight pack [128, 144] (+144 lo): WA|WB|WC at 48-col
    offsets, 45 used cols each (3 types x 15 bins on the output partition
    axis: rows 15t+q)."""
    WYf = _build_wy(sy)  # [128, 3, 15] f32
    for t in range(3):
        WYf[:, t, :] *= np.float32(max(min(float(sx[t + 1]), 1.0), 0.0))
    WA = np.zeros((128, 45), np.float32)
    WB = np.zeros((128, 45), np.float32)
    WC = np.zeros((128, 45), np.float32)
    WA[:, 0:15] = WYf[:, 0]
    WB[:, 0:15] = -16.0 * WYf[:, 0]
    WB[:, 15:30] = WYf[:, 1]
    WC[:, 15:30] = -16.0 * WYf[:, 1]
    WC[:, 30:45] = WYf[:, 2]
    hi = np.zeros((128, 144), np.float32)
    lo = np.zeros((128, 144), np.float32)
    for o, W in ((0, WA), (48, WB), (96, WC)):
        h = W.astype(np.float16).astype(np.float32)
        hi[:, o : o + 45] = h
        lo[:, o : o + 45] = W - h
    use_lo = bool(np.any(lo != 0))
    if use_lo:
        pack = np.concatenate([hi, lo], axis=1).astype(np.float16)
    else:
        pack = hi.astype(np.float16)
    return pack, use_lo


def _kernel_fast(smap, sx, sy):
    from concourse.bass_utils import run_bass_kernel_spmd

    wst, use_lo = _fast_weights(sx, sy)
    wx8 = np.zeros((128, 16), np.float32)
    for i in range(128):
        wx8[i, i // 8] = 1.0
    wx8 = wx8.astype(FP8E5)

    key = ("fast", use_lo)
    if key not in _PROG_CACHE:
        _PROG_CACHE[key] = _build_program_fast(use_lo)
    nc = _PROG_CACHE[key]

    in_maps = []
    for c in range(NCORES):
        in_maps.append(
            {
                "muT": _pack_mu(smap[512 * c : 512 * c + 512]),
                "wx8": wx8,
                "wst": wst,
            }
        )

    res = run_bass_kernel_spmd(
        nc,
        in_maps,
        core_ids=list(range(NCORES)),
        trace=bool(int(os.environ.get("KERNEL_TRACE", "0"))),
    )
    kernel._last_results = res

    full = np.empty((4, 512, 512), np.float32)
    for c in range(NCORES):
        ob = res.results[c]["outbuf"]  # [45, 35*64]
        # rows 15t+q, cols 64T+bx -> hist[t, bx, 15T+q]
        h = ob.reshape(3, 15, NT, 64).transpose(0, 3, 2, 1).reshape(3, 64, NT * 15)
        full[1:4, 64 * c : 64 * c + 64, :] = h[:, :, :512]
    full[0] = full[1]
    return full


# ======================================================================
# general fallback (previous mask-based kernel, unchanged)
# ======================================================================

def _nbins(T):
    return 15 if T < 34 else 2


def _build_wx(sx):
    """f32 [IPAD, 3, 65]: x-stage stationary.  Local row r = i - 512c; col 64
    is the spill bin (kx=1 of the last 8 rows -> next core's first bin)."""
    W = np.zeros((IPAD, 3, 65), np.float32)
    for tp in range(3):
        o0, o1 = _oxy_weights(np.float32(sx[tp + 1]))
        for r in range(512):
            W[r, tp, r // 8] += o0[r % 8]
            W[r, tp, r // 8 + 1] += o1[r % 8]
    return W


def _hi_lo(w):
    hi = w.astype(BF16)
    lo = (w - hi.astype(np.float32)).astype(BF16)
    return hi, lo


def _build_program(use_xlo, use_ylo):
    _install_wait_split()
    import os as _os
    import concourse.bass as bass
    import concourse.mybir as mybir
    from concourse.tile import TileContext
    from contextlib import ExitStack

    bufs_m = int(_os.environ.get("KERNEL_BUFS_M", "4"))
    bufs_pa = int(_os.environ.get("KERNEL_BUFS_PA", "3"))
    nmask_dve = int(_os.environ.get("KERNEL_MASK_DVE", "2"))
    ndrain_dve = int(_os.environ.get("KERNEL_DRAIN_DVE", "0"))

    dt = mybir.dt
    nc = bass.Bass()
    smapT = nc.declare_dram_parameter(
        "smapT", [128, NT * RPAD], dt.int16, isOutput=False
    )
    WTOT = 96 + 780 + (780 if use_xlo else 0)
    WPACK = nc.declare_dram_parameter(
        "wpack", [128, WTOT], dt.bfloat16, isOutput=False
    )
    OUT = nc.declare_dram_parameter("outbuf", [3, 65, 512], dt.float32, isOutput=True)

    RW = 30 if use_ylo else 15
    NCH = [sum(len(GROUPS[g]) for g in gc) for gc in GCHUNKS]

    with ExitStack() as ctx:
        tc = ctx.enter_context(TileContext(nc))
        pp = ctx.enter_context(tc.tile_pool(name="persist", bufs=1))
        mp = ctx.enter_context(tc.tile_pool(name="masks", bufs=bufs_m))
        if use_xlo:
            bufs_pa = min(bufs_pa, 2)
        pA = ctx.enter_context(tc.tile_pool(name="psA", bufs=bufs_pa, space="PSUM"))
        pB = ctx.enter_context(tc.tile_pool(name="psB", bufs=2, space="PSUM"))
        pB2 = (
            ctx.enter_context(tc.tile_pool(name="psB2", bufs=2, space="PSUM"))
            if use_xlo
            else None
        )

        wpack = pp.tile([128, WTOT], dt.bfloat16, name="wpack", tag="wpack")
        wy = wpack[:, 0:96]
        wxh = [
            [wpack[:, 96 + 65 * (4 * t + k) : 96 + 65 * (4 * t + k) + 65] for k in range(4)]
            for t in range(3)
        ]
        wxl = (
            [
                [
                    wpack[:, 876 + 65 * (4 * t + k) : 876 + 65 * (4 * t + k) + 65]
                    for k in range(4)
                ]
                for t in range(3)
            ]
            if use_xlo
            else None
        )
        AD = [
            pp.tile([96, 6 * IPAD], dt.bfloat16, name=f"ad_{i}", tag=f"ad_{i}")
            for i in range(NAD)
        ]
        ATC = [
            pp.tile([128, 384 * 12], dt.bfloat16, name=f"atc_{cc}", tag=f"atc_{cc}")
            for cc in range(len(GCHUNKS))
        ]
        stbig = pp.tile([128, NT * RPAD], dt.int16, name="stbig", tag="stbig")
        outp = [
            pp.tile([65, 512], dt.float32, name=f"outp_{t}", tag=f"outp_{t}")
            for t in range(3)
        ]

        nc.sync.dma_start(out=wpack[:, :], in_=WPACK[:, :])

        sizes = [int(x) for x in _os.environ.get("KERNEL_GRADE", "2,4,6").split(",")]
        while sum(sizes) < NT:
            sizes.append(min(8, NT - sum(sizes)))
        pos = 0
        for sz in sizes:
            c0 = pos * RPAD
            c1 = (pos + sz) * RPAD
            nc.sync.dma_start(out=stbig[:, c0:c1], in_=smapT[:, c0:c1])
            pos += sz
        drain_i = 0
        for g, Ts in enumerate(GROUPS):
            ad = AD[g % NAD]
            cc = next(i for i, gc in enumerate(GCHUNKS) if g in gc)
            for pr in range(0, len(Ts), 2):
                subs = [s for s in (0, 1) if pr + s < len(Ts)]
                psA = pA.tile([96, 1024], dt.float32)
                for s in subs:
                    jl = pr + s
                    T = Ts[jl]
                    st16 = stbig[:, RPAD * T : RPAD * T + RPAD]
                    masks = []
                    for t in range(3):
                        m = mp.tile([128, RPAD], dt.bfloat16, tag=f"m{t}")
                        use_dve = t < nmask_dve or (t == nmask_dve and (T % 2) == 0)
                        eng = nc.vector if use_dve else nc.gpsimd
                        eng.tensor_scalar(
                            m[:, :],
                            st16,
                            float(t + 1),
                            None,
                            mybir.AluOpType.is_equal,
                        )
                        masks.append(m)
                    for t in range(3):
                        nc.tensor.matmul(
                            psA[32 * t : 32 * t + 32, 512 * s : 512 * s + 512],
                            lhsT=wy[:, 32 * t : 32 * t + 32],
                            rhs=masks[t][:, :],
                            start=True,
                            stop=True,
                        )
                w = 512 * len(subs)
                if (drain_i % 3) < ndrain_dve:
                    nc.vector.tensor_copy(
                        out=ad[0:96, IPAD * pr : IPAD * pr + w], in_=psA[:, 0:w]
                    )
                else:
                    nc.scalar.copy(ad[0:96, IPAD * pr : IPAD * pr + w], psA[:, 0:w])
                drain_i += 1
            nT = len(Ts)
            jlo = GOFF[g]
            nc.sync.dma_start_transpose(
                ATC[cc][:, 384 * jlo : 384 * jlo + 384 * nT].rearrange(
                    "p (c q) -> p c q", q=96
                ),
                ad[0:96, 0 : nT * IPAD],
            )

        for cc, gc in enumerate(GCHUNKS):
            for t in range(3):
                njl = NCH[cc]
                psB = pB.tile([65, 512], dt.float32)
                for k in range(4):
                    rhs = ATC[cc][:, 0 : 384 * njl].rearrange(
                        "p (jl four q) -> p jl four q", four=4, q=96
                    )[:, :, k, 32 * t : 32 * t + RW]
                    nc.tensor.matmul(
                        psB[:, 0 : RW * njl],
                        lhsT=wxh[t][k],
                        rhs=rhs,
                        start=(k == 0),
                        stop=(k == 3),
                    )
                psB2 = None
                if use_xlo:
                    psB2 = pB2.tile([65, 512], dt.float32)
                    for k in range(4):
                        rhs = ATC[cc][:, 0 : 384 * njl].rearrange(
                            "p (jl four q) -> p jl four q", four=4, q=96
                        )[:, :, k, 32 * t : 32 * t + 15]
                        nc.tensor.matmul(
                            psB2[:, 0 : 15 * njl],
                            lhsT=wxl[t][k],
                            rhs=rhs,
                            start=(k == 0),
                            stop=(k == 3),
                        )
                reg = psB[:, 0 : RW * njl].rearrange("p (jl c) -> p jl c", c=RW)
                if use_ylo:
                    nc.vector.scalar_tensor_tensor(
                        out=reg[:, :, 0:15],
                        in0=reg[:, :, 0:15],
                        scalar=1.0,
                        in1=reg[:, :, 15:30],
                        op0=mybir.AluOpType.mult,
                        op1=mybir.AluOpType.add,
                    )
                if use_xlo:
                    nc.vector.scalar_tensor_tensor(
                        out=reg[:, :, 0:15],
                        in0=reg[:, :, 0:15],
                        scalar=1.0,
                        in1=psB2[:, 0 : 15 * njl].rearrange(
                            "p (jl c) -> p jl c", c=15
                        ),
                        op0=mybir.AluOpType.mult,
                        op1=mybir.AluOpType.add,
                    )
                T0 = GROUPS[gc[0]][0]
                nfull = njl if cc < 2 else njl - 1
                nc.scalar.activation(
                    outp[t][:, 15 * T0 : 15 * T0 + 15 * nfull],
                    reg[:, 0:nfull, 0:15],
                    mybir.ActivationFunctionType.Copy,
                    bias=64.0,
                    scale=-1.0,
                )
                if cc == 2:
                    nc.scalar.activation(
                        outp[t][:, 510:512],
                        reg[:, njl - 1, 0:2],
                        mybir.ActivationFunctionType.Copy,
                        bias=64.0,
                        scale=-1.0,
                    )
                    nc.sync.dma_start(out=OUT[t, :, :], in_=outp[t][:, :])
    return nc


def _kernel_general(smap, sx, sy):
    from concourse.bass_utils import run_bass_kernel_spmd

    WYf = _build_wy(sy)
    WXf = _build_wx(sx)
    wy_hi, wy_lo = _hi_lo(WYf)
    wx_hi, wx_lo = _hi_lo(WXf)
    use_ylo = bool(np.any(wy_lo.astype(np.float32) != 0))
    use_xlo = bool(np.any(wx_lo.astype(np.float32) != 0))

    WTOT = 96 + 780 + (780 if use_xlo else 0)
    wpk = np.zeros((128, WTOT), BF16)
    for t in range(3):
        wpk[:, 32 * t : 32 * t + 15] = wy_hi[:, t, :]
        wpk[:, 32 * t + 15 : 32 * t + 30] = wy_lo[:, t, :]
        for k in range(4):
            o = 96 + 65 * (4 * t + k)
            wpk[:, o : o + 65] = wx_hi[128 * k : 128 * k + 128, t, :]
            if use_xlo:
                o2 = 876 + 65 * (4 * t + k)
                wpk[:, o2 : o2 + 65] = wx_lo[128 * k : 128 * k + 128, t, :]

    key = (use_xlo, use_ylo)
    if key not in _PROG_CACHE:
        _PROG_CACHE[key] = _build_program(use_xlo, use_ylo)
    nc = _PROG_CACHE[key]

    in_maps = []
    for c in range(NCORES):
        sjp = np.zeros((JPAD, RPAD), np.int16)
        sjp[8 : 8 + 4096, :] = smap[512 * c : 512 * c + 512].T
        big = np.empty((128, NT * RPAD), np.int16)
        for T in range(NT):
            big[:, RPAD * T : RPAD * T + RPAD] = sjp[120 * T : 120 * T + 128, :]
        in_maps.append({"smapT": big, "wpack": wpk})

    res = run_bass_kernel_spmd(
        nc,
        in_maps,
        core_ids=list(range(NCORES)),
        trace=bool(int(os.environ.get("KERNEL_TRACE", "0"))),
    )
    kernel._last_results = res

    full = np.empty((4, 512, 512), np.float32)
    for c in range(NCORES):
        ob = res.results[c]["outbuf"]
        full[1:4, 64 * c : 64 * c + 64, :] = ob[:, 0:64, :]
    for c in range(NCORES - 1):
        full[1:4, 64 * (c + 1), :] += res.results[c]["outbuf"][:, 64, :] - np.float32(
            64.0
        )
    full[0] = full[1]
    return full


def kernel(site_type_map, site_size_x, site_size_y):
    smap = np.asarray(site_type_map, dtype=np.int32)
    sx = np.asarray(site_size_x, dtype=np.float32)
    sy = np.asarray(site_size_y, dtype=np.float32)

    if bool(np.all(sx[1:4] <= 1.0)):
        return _kernel_fast(smap, sx, sy)
    return _kernel_general(smap, sx, sy)
